# revision 22
# baseline (speedup 1.0000x reference)
"""FFTConv2d kernel for trn2, 8 NeuronCores.

Math: reference einsum 'bchw,oihw->bohw' factorizes:
  Y[b,o] = conv_same(sum_c x[b,c], flip(sum_i w[o,i])) + bias[o]
i.e. a single-channel 3x3 "same" convolution per (b,o) pair.
bias is added on the host (it is a [64] vector on a [16,64,128,128]
output; negligible), so no ones/bias row rides the matmul.

Per core (2 batches), all SBUF data fp16 (PSUM accum fp32):
  1. xin [128 (b,c), 16384] <- x fp16, 9 HBM DMA pieces (SP ring),
     emitted first; cs chunks chase the pieces (input ~11.7us is the
     cs-phase wall at ~360GB/s).
  2. A dozen tiny warm-up matmuls keep PE busy early so the p-state
     ramp (2.4GHz after ~3us continuous) completes before the real
     stream starts.
  3. Channel-sum: ones-indicator matmul pairs -> PSUM [2, 512];
     FD=512 copies (DVE/Act alternating) drain 4 image rows into the
     padded staging = P9 partitions {0,1} (row stride 130, zero
     borders memset once).
  4. P9 [18, 16902], partition p=6jj+2ip+b holds staging shifted by
     130jj+ip; p=0,1 IS the staging; shifts built via a DRAM bounce
     (SBUF->SBUF DMA is ~5x slower per byte than HBM paths): 5
     staging segments written to an Internal HBM scratch as they
     drain (gpsimd/SWDGE ring), then per-segment shifted reads (3
     DMAs, one per row-shift jj) rebuild the 16 shifted partitions.
     The last sliver skips the bounce (direct SBUF->SBUF, one hop)
     to shorten the post-cs critical path.
  5. Conv: 33 flat 512-col chunks; K=18 fp16 matmuls into a 4-deep
     PSUM rotation; FD=512 copies -> yt. Conv chunks are emitted
     interleaved into the cs stream once their staging segment is
     expected, soaking up PE idle while cs waits on input DMA.
  6. yt [128, 16640] -> HBM in 9 pieces on the SP ring (7x2080 +
     2x1040 so the final piece is small); host strips the 2 junk
     cols per 130-wide row, upconverts to fp32, adds bias.
"""

import os
import sys
from functools import lru_cache

import numpy as np

for _p in ("/opt/trn_rl_repo", "/root/.axon_site/_ro/trn_rl_repo"):
    if os.path.isdir(_p) and _p not in sys.path:
        sys.path.insert(0, _p)

B, CIN, COUT, H, W = 16, 64, 64, 128, 128
N_CORES = 8
BPC = B // N_CORES  # 2
NPART = BPC * CIN  # 128
NOUT = BPC * COUT  # 128
WROW = W + 2  # 130
HW = H * W  # 16384
HHW = H * WROW  # 16640 (130-wide output rows)
LSP = (H + 2) * WROW + 2  # 16902 (padded staging length)
NK = BPC * 9  # 18
NCV = 33  # conv chunks: 32x512 + 1x256
NWARM = 5

# staging segment g is in scratch after cs chunk GCH[g] drains
# (chunk c covers staging positions < (4c+5)*130 + 1 incl. borders);
# shifted read g rebuilds p9 dst positions [RSEG[g], RSEG[g+1])
# (needs src to dst_end + 262). Final sliver [RSEG[5], 16640) goes
# direct SBUF->SBUF after the last drain.
GCH = [3, 9, 15, 23, 31]
PSEG = [0] + [(4 * c + 5) * WROW + 1 for c in GCH[:-1]] + [LSP]
RSEG = [0] + [PSEG[g + 1] - 262 for g in range(4)] + [HHW]
# conv chunks unlocked by read g (chunk j needs dst < 512j+512):
#   g0: 0-2, g1: 3-8, g2: 9-14, g3: 15-23, g4: 24-32
# conv groups are emitted into the cs stream at points where their
# read group is expected (measured: group ready ~= drain(GCH[g])+6us)
CONV_EMIT = {26: range(0, 3), 28: range(3, 9), 30: range(9, 15)}


@lru_cache(maxsize=1)
def _build():
    import concourse.bacc as bacc
    import concourse.mybir as mybir
    import concourse.tile as tile
    from concourse.ap import AP

    f32 = mybir.dt.float32
    f16 = mybir.dt.float16

    nc = bacc.Bacc("TRN2", target_bir_lowering=False, debug=False, num_devices=N_CORES)

    xh = nc.dram_tensor("xh", [NPART, HW], f16, kind="ExternalInput")
    wbh = nc.dram_tensor("wb", [NK, NOUT], f16, kind="ExternalInput")
    y = nc.dram_tensor("y", [NOUT, HHW], f16, kind="ExternalOutput")
    scratch = nc.dram_tensor("xs_scratch", [BPC, LSP], f16, kind="Internal")
    dump = os.environ.get("KDUMP")
    if dump:
        p9_d = nc.dram_tensor("p9_d", [NK, HHW], f16, kind="ExternalOutput")

    with tile.TileContext(nc) as tc:
        with (
            tc.tile_pool(name="main", bufs=1) as mp,
            tc.tile_pool(name="ps", bufs=1, space="PSUM") as ps_pool,
        ):
            xin = mp.tile([NPART, HW], f16, tag="xin")
            p9 = mp.tile([NK, LSP], f16, tag="p9")
            yt = mp.tile([NOUT, HHW], f16, tag="yt")
            ones_t = mp.tile([NPART, BPC], f16, tag="ones_t")
            wb_t = mp.tile([NK, NOUT], f16, tag="wb")

            p9t = p9.tensor

            csb = [
                ps_pool.tile([BPC, 512], f32, tag=f"cs{i}", name=f"cs{i}")
                for i in range(4)
            ]
            cvb = [
                ps_pool.tile([NOUT, 512], f32, tag=f"cv{i}", name=f"cv{i}")
                for i in range(4)
            ]

            # input first on the SP (HWDGE) ring, which alone sustains
            # ~360GB/s; fine-grained early pieces so cs chunks start with
            # minimal piece-boundary (completion-semaphore) quantization
            pieces = [(0, 1024), (1024, 1024)] + [
                (2048 * q, 2048) for q in range(1, 8)
            ]
            for o, n in pieces:
                nc.sync.dma_start(out=xin[:, o : o + n], in_=xh.ap()[:, o : o + n])

            # weights via the Act HWDGE ring (one early trigger; keeps the
            # gpsimd/SWDGE queue free for the shifted reads)
            nc.scalar.dma_start(out=wb_t[:, :], in_=wbh.ap()[:, :])

            # ones indicator [128, 2]: col b is 1 for partitions of batch b
            nc.vector.memset(ones_t[0:CIN, 0:1], 1.0)
            nc.vector.memset(ones_t[0:CIN, 1:2], 0.0)
            nc.vector.memset(ones_t[CIN:NPART, 0:1], 0.0)
            nc.vector.memset(ones_t[CIN:NPART, 1:2], 1.0)

            # staging zero borders in P9 partitions {0, 1}:
            # row -1, row 128 + tail, and (right col, next left col) pairs
            nc.vector.memset(
                AP(tensor=p9t, offset=0, ap=[[LSP, BPC], [1, WROW]]), 0.0
            )
            nc.vector.memset(
                AP(
                    tensor=p9t,
                    offset=(H + 1) * WROW,
                    ap=[[LSP, BPC], [1, LSP - (H + 1) * WROW]],
                ),
                0.0,
            )
            nc.vector.memset(
                AP(
                    tensor=p9t,
                    offset=WROW - 1,
                    ap=[[LSP, BPC], [WROW, H + 1], [1, 2]],
                ),
                0.0,
            )

            # PE p-state warm-up: 512-row garbage matmuls (moving = yt,
            # which nothing has written yet; out = csb[3], first really
            # produced by cs chunk 3) keep PE continuously busy from ~8us
            # until the input stream arrives, so the 2.4GHz ramp completes
            # before the real stream
            for _ in range(NWARM):
                nc.tensor.matmul(
                    csb[3][:, :],
                    ones_t[:, :],
                    yt[:, 0:512],
                    start=True,
                    stop=True,
                )

            copy_engines = [nc.vector, nc.scalar]

            def ecopy(idx, dst, src):
                eng = copy_engines[idx % 2]
                if eng is nc.vector:
                    eng.tensor_copy(dst, src)
                else:
                    eng.copy(dst, src)

            def emit_cs(k):
                # ones-matmul of 512 cols (4 rows) -> [2, 512]; 1 copy
                ps = csb[k % 4]
                pst = ps.tensor
                nc.tensor.matmul(
                    ps[:, :],
                    ones_t[:, :],
                    xin[:, 512 * k : 512 * k + 512],
                    start=True,
                    stop=True,
                )
                dst = AP(
                    tensor=p9t,
                    offset=(4 * k + 1) * WROW + 1,
                    ap=[[LSP, BPC], [WROW, 4], [1, W]],
                )
                src = AP(
                    tensor=pst, offset=0, ap=[[512, BPC], [W, 4], [1, W]]
                )
                ecopy(k, dst, src)

            def shifted_reads(src_t, src_pitch, r0, r1, eng):
                # rebuild p9 parts 2..17 for dst positions [r0, r1) from a
                # staging image at src_t (partition pitch src_pitch):
                # 3 DMAs, one per row-shift jj; jj=0 skips ip=0 (= staging)
                ln = r1 - r0
                eng.dma_start(
                    out=AP(
                        tensor=p9t,
                        offset=2 * LSP + r0,
                        ap=[[LSP, 4], [1, ln]],
                    ),
                    in_=AP(
                        tensor=src_t,
                        offset=r0 + 1,
                        ap=[[1, 2], [src_pitch, BPC], [1, ln]],
                    ),
                )
                for jj in (1, 2):
                    eng.dma_start(
                        out=AP(
                            tensor=p9t,
                            offset=6 * jj * LSP + r0,
                            ap=[[LSP, 6], [1, ln]],
                        ),
                        in_=AP(
                            tensor=src_t,
                            offset=r0 + WROW * jj,
                            ap=[[1, 3], [src_pitch, BPC], [1, ln]],
                        ),
                    )

            def emit_bounce(g):
                # staging segment -> HBM scratch on the Act HWDGE ring:
                # GCH are odd chunks, whose drains run on Act, so the write
                # trigger sits right behind its gating drain in Act's own
                # stream (no cross-engine sem hop, empty FIFO). Shifted
                # reads go on the gpsimd/SWDGE ring.
                o0, o1 = PSEG[g], PSEG[g + 1]
                nc.scalar.dma_start(
                    out=scratch.ap()[:, o0:o1],
                    in_=AP(tensor=p9t, offset=o0, ap=[[LSP, BPC], [1, o1 - o0]]),
                )
                shifted_reads(
                    scratch.ap().tensor, LSP, RSEG[g], RSEG[g + 1], nc.gpsimd
                )

            def emit_conv(j):
                cv = cvb[j % 4]
                nn = 512 if j < NCV - 1 else 256
                nc.tensor.matmul(
                    cv[:, :nn],
                    wb_t[:, :],
                    p9[:, 512 * j : 512 * j + nn],
                    start=True,
                    stop=True,
                )
                ecopy(j, yt[:, 512 * j : 512 * j + nn], cv[:, :nn])

            # out pieces: 7x2080 + 2x1040; piece q ready after conv chunk
            OUT_PIECES = [(2080 * q, 2080) for q in range(7)] + [
                (14560, 1040),
                (15600, 1040),
            ]
            out_after = {4: 0, 8: 1, 12: 2, 16: 3, 20: 4, 24: 5, 28: 6, 30: 7, 32: 8}

            def emit_out(q):
                o, n = OUT_PIECES[q]
                nc.sync.dma_start(
                    out=y.ap()[:, o : o + n], in_=yt[:, o : o + n]
                )

            def emit_conv_full(j):
                emit_conv(j)
                if j in out_after:
                    emit_out(out_after[j])

            emitted = 0
            for k in range(32):
                emit_cs(k)
                if k in GCH:
                    emit_bounce(GCH.index(k))
                if k in CONV_EMIT:
                    for j in CONV_EMIT[k]:
                        emit_conv_full(j)
                        emitted = j + 1
            for j in range(emitted, NCV):
                emit_conv_full(j)
            if dump:
                nc.sync.dma_start(out=p9_d.ap()[:, :], in_=p9[:, 0:HHW])

    nc.compile()
    return nc


def _host_prep(x, weight):
    wsum = weight.sum(axis=1)  # [COUT, 3, 3]
    wb = np.zeros((NK, NOUT), np.float32)
    for b in range(BPC):
        for jj in range(3):
            for ip in range(3):
                wb[6 * jj + 2 * ip + b, b * COUT : (b + 1) * COUT] = wsum[
                    :, 2 - jj, 2 - ip
                ]
    wb = wb.astype(np.float16)

    in_maps = []
    for r in range(N_CORES):
        xhr = np.ascontiguousarray(
            x[r * BPC : (r + 1) * BPC].reshape(NPART, HW)
        ).astype(np.float16)
        in_maps.append({"xh": xhr, "wb": wb})
    return in_maps


def kernel(x, weight, bias):
    from concourse.bass_utils import run_bass_kernel_spmd

    x = np.asarray(x)
    weight = np.asarray(weight)
    bias = np.asarray(bias)
    nc = _build()
    in_maps = _host_prep(x, weight)
    res = run_bass_kernel_spmd(nc, in_maps, core_ids=list(range(N_CORES)))
    out = np.concatenate(
        [
            np.asarray(res.results[r]["y"])
            .astype(np.float32)
            .reshape(BPC, COUT, H, WROW)[:, :, :, :W]
            for r in range(N_CORES)
        ],
        axis=0,
    )
    return out + bias.astype(np.float32)[None, :, None, None]


# revision 24
# speedup vs baseline: 1.0936x; 1.0936x over previous
"""FFTConv2d kernel for trn2, 8 NeuronCores.

Math: reference einsum 'bchw,oihw->bohw' factorizes:
  Y[b,o] = conv_same(sum_c x[b,c], flip(sum_i w[o,i])) + bias[o]
i.e. a single-channel 3x3 "same" convolution per (b,o) pair.
bias is added on the host (it is a [64] vector on a [16,64,128,128]
output; negligible), so no ones/bias row rides the matmul.

Per core (2 batches), all SBUF data fp16 (PSUM accum fp32):
  1. xin [128 (b,c), 16384] <- x fp16, 9 HBM DMA pieces (SP ring),
     emitted first; cs chunks chase the pieces (input ~11.7us is the
     cs-phase wall at ~360GB/s).
  2. A dozen tiny warm-up matmuls keep PE busy early so the p-state
     ramp (2.4GHz after ~3us continuous) completes before the real
     stream starts.
  3. Channel-sum: ones-indicator matmul pairs -> PSUM [2, 512];
     FD=512 copies (DVE/Act alternating) drain 4 image rows into the
     padded staging = P9 partitions {0,1} (row stride 130, zero
     borders memset once).
  4. P9 [18, 16902], partition p=6jj+2ip+b holds staging shifted by
     130jj+ip; p=0,1 IS the staging; shifts built via a DRAM bounce
     (SBUF->SBUF DMA is ~5x slower per byte than HBM paths): 5
     staging segments written to an Internal HBM scratch as they
     drain (gpsimd/SWDGE ring), then per-segment shifted reads (3
     DMAs, one per row-shift jj) rebuild the 16 shifted partitions.
     The last sliver skips the bounce (direct SBUF->SBUF, one hop)
     to shorten the post-cs critical path.
  5. Conv: 33 flat 512-col chunks; K=18 fp16 matmuls into a 4-deep
     PSUM rotation; FD=512 copies -> yt. Conv chunks are emitted
     interleaved into the cs stream once their staging segment is
     expected, soaking up PE idle while cs waits on input DMA.
  6. yt [128, 16640] -> HBM in 9 pieces on the SP ring (7x2080 +
     2x1040 so the final piece is small); host strips the 2 junk
     cols per 130-wide row, upconverts to fp32, adds bias.
"""

import os
import sys
from functools import lru_cache

import numpy as np

for _p in ("/opt/trn_rl_repo", "/root/.axon_site/_ro/trn_rl_repo"):
    if os.path.isdir(_p) and _p not in sys.path:
        sys.path.insert(0, _p)

B, CIN, COUT, H, W = 16, 64, 64, 128, 128
N_CORES = 8
BPC = B // N_CORES  # 2
NPART = BPC * CIN  # 128
NOUT = BPC * COUT  # 128
WROW = W + 2  # 130
HW = H * W  # 16384
HHW = H * WROW  # 16640 (130-wide output rows)
LSP = (H + 2) * WROW + 2  # 16902 (padded staging length)
NK = BPC * 9  # 18
NCV = 33  # conv chunks: 32x512 + 1x256
NWARM = 5

# staging segment g is in scratch after cs chunk GCH[g] drains
# (chunk c covers staging positions < (4c+5)*130 + 1 incl. borders);
# shifted read g rebuilds p9 dst positions [RSEG[g], RSEG[g+1])
# (needs src to dst_end + 262). Final sliver [RSEG[5], 16640) goes
# direct SBUF->SBUF after the last drain.
GCH = [7, 15, 23, 31]
PSEG = [0] + [(4 * c + 5) * WROW + 1 for c in GCH[:-1]] + [LSP]
RSEG = [0] + [PSEG[g + 1] - 262 for g in range(3)] + [HHW]
# conv chunks unlocked by read g (chunk j needs dst < 512j+512):
#   g0: 0-6, g1: 7-14, g2: 15-23, g3: 24-32
# strict cs -> conv phases: every read group lands well before the
# in-order conv train reaches its chunks (interleaving conv into the
# cs stream measurably backfires: one late read stalls PE for all
# downstream work)


@lru_cache(maxsize=1)
def _build():
    import concourse.bacc as bacc
    import concourse.mybir as mybir
    import concourse.tile as tile
    from concourse.ap import AP

    f32 = mybir.dt.float32
    f16 = mybir.dt.float16

    nc = bacc.Bacc("TRN2", target_bir_lowering=False, debug=False, num_devices=N_CORES)

    xh = nc.dram_tensor("xh", [NPART, HW], f16, kind="ExternalInput")
    wbh = nc.dram_tensor("wb", [NK, NOUT], f16, kind="ExternalInput")
    y = nc.dram_tensor("y", [NOUT, HHW], f16, kind="ExternalOutput")
    scratch = nc.dram_tensor("xs_scratch", [BPC, LSP], f16, kind="Internal")
    dump = os.environ.get("KDUMP")
    if dump:
        p9_d = nc.dram_tensor("p9_d", [NK, HHW], f16, kind="ExternalOutput")

    with tile.TileContext(nc) as tc:
        with (
            tc.tile_pool(name="main", bufs=1) as mp,
            tc.tile_pool(name="ps", bufs=1, space="PSUM") as ps_pool,
        ):
            xin = mp.tile([NPART, HW], f16, tag="xin")
            p9 = mp.tile([NK, LSP], f16, tag="p9")
            yt = mp.tile([NOUT, HHW], f16, tag="yt")
            ones_t = mp.tile([NPART, BPC], f16, tag="ones_t")
            wb_t = mp.tile([NK, NOUT], f16, tag="wb")

            p9t = p9.tensor

            csb = [
                ps_pool.tile([BPC, 512], f32, tag=f"cs{i}", name=f"cs{i}")
                for i in range(4)
            ]
            cvb = [
                ps_pool.tile([NOUT, 512], f32, tag=f"cv{i}", name=f"cv{i}")
                for i in range(4)
            ]

            # input first on the SP (HWDGE) ring, which alone sustains
            # ~360GB/s; fine-grained early pieces so cs chunks start with
            # minimal piece-boundary (completion-semaphore) quantization
            pieces = [(0, 1024), (1024, 1024)] + [
                (2048 * q, 2048) for q in range(1, 8)
            ]
            for o, n in pieces:
                nc.sync.dma_start(out=xin[:, o : o + n], in_=xh.ap()[:, o : o + n])

            # weights via the Act HWDGE ring (one early trigger; keeps the
            # gpsimd/SWDGE queue free for the shifted reads)
            nc.scalar.dma_start(out=wb_t[:, :], in_=wbh.ap()[:, :])

            # ones indicator [128, 2]: col b is 1 for partitions of batch b
            nc.vector.memset(ones_t[0:CIN, 0:1], 1.0)
            nc.vector.memset(ones_t[0:CIN, 1:2], 0.0)
            nc.vector.memset(ones_t[CIN:NPART, 0:1], 0.0)
            nc.vector.memset(ones_t[CIN:NPART, 1:2], 1.0)

            # staging zero borders in P9 partitions {0, 1}:
            # row -1, row 128 + tail, and (right col, next left col) pairs
            nc.vector.memset(
                AP(tensor=p9t, offset=0, ap=[[LSP, BPC], [1, WROW]]), 0.0
            )
            nc.vector.memset(
                AP(
                    tensor=p9t,
                    offset=(H + 1) * WROW,
                    ap=[[LSP, BPC], [1, LSP - (H + 1) * WROW]],
                ),
                0.0,
            )
            nc.vector.memset(
                AP(
                    tensor=p9t,
                    offset=WROW - 1,
                    ap=[[LSP, BPC], [WROW, H + 1], [1, 2]],
                ),
                0.0,
            )

            # PE p-state warm-up: 512-row garbage matmuls (moving = yt,
            # which nothing has written yet; out = csb[3], first really
            # produced by cs chunk 3) keep PE continuously busy from ~8us
            # until the input stream arrives, so the 2.4GHz ramp completes
            # before the real stream
            for _ in range(NWARM):
                nc.tensor.matmul(
                    csb[3][:, :],
                    ones_t[:, :],
                    yt[:, 0:512],
                    start=True,
                    stop=True,
                )

            copy_engines = [nc.vector, nc.scalar]

            def ecopy(idx, dst, src):
                eng = copy_engines[idx % 2]
                if eng is nc.vector:
                    eng.tensor_copy(dst, src)
                else:
                    eng.copy(dst, src)

            def emit_cs(k):
                # ones-matmul of 512 cols (4 rows) -> [2, 512]; 1 copy
                ps = csb[k % 4]
                pst = ps.tensor
                nc.tensor.matmul(
                    ps[:, :],
                    ones_t[:, :],
                    xin[:, 512 * k : 512 * k + 512],
                    start=True,
                    stop=True,
                )
                dst = AP(
                    tensor=p9t,
                    offset=(4 * k + 1) * WROW + 1,
                    ap=[[LSP, BPC], [WROW, 4], [1, W]],
                )
                src = AP(
                    tensor=pst, offset=0, ap=[[512, BPC], [W, 4], [1, W]]
                )
                ecopy(k, dst, src)

            def shifted_reads(src_t, src_pitch, r0, r1, eng):
                # rebuild p9 parts 2..17 for dst positions [r0, r1) from a
                # staging image at src_t (partition pitch src_pitch):
                # 3 DMAs, one per row-shift jj; jj=0 skips ip=0 (= staging)
                ln = r1 - r0
                eng.dma_start(
                    out=AP(
                        tensor=p9t,
                        offset=2 * LSP + r0,
                        ap=[[LSP, 4], [1, ln]],
                    ),
                    in_=AP(
                        tensor=src_t,
                        offset=r0 + 1,
                        ap=[[1, 2], [src_pitch, BPC], [1, ln]],
                    ),
                )
                for jj in (1, 2):
                    eng.dma_start(
                        out=AP(
                            tensor=p9t,
                            offset=6 * jj * LSP + r0,
                            ap=[[LSP, 6], [1, ln]],
                        ),
                        in_=AP(
                            tensor=src_t,
                            offset=r0 + WROW * jj,
                            ap=[[1, 3], [src_pitch, BPC], [1, ln]],
                        ),
                    )

            def emit_bounce(g):
                # staging segment -> HBM scratch on the Act HWDGE ring:
                # GCH are odd chunks, whose drains run on Act, so the write
                # trigger sits right behind its gating drain in Act's own
                # stream (no cross-engine sem hop, empty FIFO). Shifted
                # reads go on the gpsimd/SWDGE ring.
                o0, o1 = PSEG[g], PSEG[g + 1]
                nc.scalar.dma_start(
                    out=scratch.ap()[:, o0:o1],
                    in_=AP(tensor=p9t, offset=o0, ap=[[LSP, BPC], [1, o1 - o0]]),
                )
                shifted_reads(
                    scratch.ap().tensor, LSP, RSEG[g], RSEG[g + 1], nc.gpsimd
                )

            def emit_conv(j):
                cv = cvb[j % 4]
                nn = 512 if j < NCV - 1 else 256
                nc.tensor.matmul(
                    cv[:, :nn],
                    wb_t[:, :],
                    p9[:, 512 * j : 512 * j + nn],
                    start=True,
                    stop=True,
                )
                ecopy(j, yt[:, 512 * j : 512 * j + nn], cv[:, :nn])

            # out pieces: 7x2080 + 2x1040; piece q ready after conv chunk
            OUT_PIECES = [(2080 * q, 2080) for q in range(7)] + [
                (14560, 1040),
                (15600, 1040),
            ]
            out_after = {4: 0, 8: 1, 12: 2, 16: 3, 20: 4, 24: 5, 28: 6, 30: 7, 32: 8}

            def emit_out(q):
                o, n = OUT_PIECES[q]
                nc.sync.dma_start(
                    out=y.ap()[:, o : o + n], in_=yt[:, o : o + n]
                )

            def emit_conv_full(j):
                emit_conv(j)
                if j in out_after:
                    emit_out(out_after[j])

            for k in range(32):
                emit_cs(k)
                if k in GCH:
                    emit_bounce(GCH.index(k))
            for j in range(NCV):
                emit_conv_full(j)
            if dump:
                nc.sync.dma_start(out=p9_d.ap()[:, :], in_=p9[:, 0:HHW])

    nc.compile()
    return nc


def _host_prep(x, weight):
    wsum = weight.sum(axis=1)  # [COUT, 3, 3]
    wb = np.zeros((NK, NOUT), np.float32)
    for b in range(BPC):
        for jj in range(3):
            for ip in range(3):
                wb[6 * jj + 2 * ip + b, b * COUT : (b + 1) * COUT] = wsum[
                    :, 2 - jj, 2 - ip
                ]
    wb = wb.astype(np.float16)

    in_maps = []
    for r in range(N_CORES):
        xhr = np.ascontiguousarray(
            x[r * BPC : (r + 1) * BPC].reshape(NPART, HW)
        ).astype(np.float16)
        in_maps.append({"xh": xhr, "wb": wb})
    return in_maps


def kernel(x, weight, bias):
    from concourse.bass_utils import run_bass_kernel_spmd

    x = np.asarray(x)
    weight = np.asarray(weight)
    bias = np.asarray(bias)
    nc = _build()
    in_maps = _host_prep(x, weight)
    res = run_bass_kernel_spmd(nc, in_maps, core_ids=list(range(N_CORES)))
    out = np.concatenate(
        [
            np.asarray(res.results[r]["y"])
            .astype(np.float32)
            .reshape(BPC, COUT, H, WROW)[:, :, :, :W]
            for r in range(N_CORES)
        ],
        axis=0,
    )
    return out + bias.astype(np.float32)[None, :, None, None]


# revision 27
# speedup vs baseline: 1.0952x; 1.0015x over previous
"""FFTConv2d kernel for trn2, 8 NeuronCores.

Math: reference einsum 'bchw,oihw->bohw' factorizes:
  Y[b,o] = conv_same(sum_c x[b,c], flip(sum_i w[o,i])) + bias[o]
i.e. a single-channel 3x3 "same" convolution per (b,o) pair.
bias is added on the host (it is a [64] vector on a [16,64,128,128]
output; negligible), so no ones/bias row rides the matmul.

Per core (2 batches), all SBUF data fp16 (PSUM accum fp32):
  1. xin [128 (b,c), 16384] <- x fp16, 9 HBM DMA pieces (SP ring),
     emitted first; cs chunks chase the pieces (input ~11.7us is the
     cs-phase wall at ~360GB/s).
  2. A dozen tiny warm-up matmuls keep PE busy early so the p-state
     ramp (2.4GHz after ~3us continuous) completes before the real
     stream starts.
  3. Channel-sum: ones-indicator matmul pairs -> PSUM [2, 512];
     FD=512 copies (DVE/Act alternating) drain 4 image rows into the
     padded staging = P9 partitions {0,1} (row stride 130, zero
     borders memset once).
  4. P9 [18, 16902], partition p=6jj+2ip+b holds staging shifted by
     130jj+ip; p=0,1 IS the staging; shifts built via a DRAM bounce
     (SBUF->SBUF DMA is ~5x slower per byte than HBM paths): 5
     staging segments written to an Internal HBM scratch as they
     drain (gpsimd/SWDGE ring), then per-segment shifted reads (3
     DMAs, one per row-shift jj) rebuild the 16 shifted partitions.
     The last sliver skips the bounce (direct SBUF->SBUF, one hop)
     to shorten the post-cs critical path.
  5. Conv: 33 flat 512-col chunks; K=18 fp16 matmuls into a 4-deep
     PSUM rotation; FD=512 copies -> yt. Conv chunks are emitted
     interleaved into the cs stream once their staging segment is
     expected, soaking up PE idle while cs waits on input DMA.
  6. yt [128, 16640] -> HBM in 9 pieces on the SP ring (7x2080 +
     2x1040 so the final piece is small); host strips the 2 junk
     cols per 130-wide row, upconverts to fp32, adds bias.
"""

import os
import sys
from functools import lru_cache

import numpy as np

for _p in ("/opt/trn_rl_repo", "/root/.axon_site/_ro/trn_rl_repo"):
    if os.path.isdir(_p) and _p not in sys.path:
        sys.path.insert(0, _p)

B, CIN, COUT, H, W = 16, 64, 64, 128, 128
N_CORES = 8
BPC = B // N_CORES  # 2
NPART = BPC * CIN  # 128
NOUT = BPC * COUT  # 128
WROW = W + 2  # 130
HW = H * W  # 16384
HHW = H * WROW  # 16640 (130-wide output rows)
LSP = (H + 2) * WROW + 2  # 16902 (padded staging length)
NK = BPC * 9  # 18
NCV = 33  # conv chunks: 32x512 + 1x256
NWARM = 4

# staging segment g is in scratch after cs chunk GCH[g] drains
# (chunk c covers staging positions < (4c+5)*130 + 1 incl. borders);
# shifted read g rebuilds p9 dst positions [RSEG[g], RSEG[g+1])
# (needs src to dst_end + 262). Final sliver [RSEG[5], 16640) goes
# direct SBUF->SBUF after the last drain.
GCH = [7, 15, 23, 31]
PSEG = [0] + [(4 * c + 5) * WROW + 1 for c in GCH[:-1]] + [LSP]
RSEG = [0] + [PSEG[g + 1] - 262 for g in range(3)] + [HHW]
# conv chunks unlocked by read g (chunk j needs dst < 512j+512):
#   g0: 0-6, g1: 7-14, g2: 15-23, g3: 24-32
# strict cs -> conv phases: every read group lands well before the
# in-order conv train reaches its chunks (interleaving conv into the
# cs stream measurably backfires: one late read stalls PE for all
# downstream work)


@lru_cache(maxsize=1)
def _build():
    import concourse.bacc as bacc
    import concourse.mybir as mybir
    import concourse.tile as tile
    from concourse.ap import AP

    f32 = mybir.dt.float32
    f16 = mybir.dt.float16

    nc = bacc.Bacc("TRN2", target_bir_lowering=False, debug=False, num_devices=N_CORES)

    xh = nc.dram_tensor("xh", [NPART, HW], f16, kind="ExternalInput")
    wbh = nc.dram_tensor("wb", [NK, NOUT], f16, kind="ExternalInput")
    y = nc.dram_tensor("y", [NOUT, HHW], f16, kind="ExternalOutput")
    scratch = nc.dram_tensor("xs_scratch", [BPC, LSP], f16, kind="Internal")
    dump = os.environ.get("KDUMP")
    if dump:
        p9_d = nc.dram_tensor("p9_d", [NK, HHW], f16, kind="ExternalOutput")

    with tile.TileContext(nc) as tc:
        with (
            tc.tile_pool(name="main", bufs=1) as mp,
            tc.tile_pool(name="ps", bufs=1, space="PSUM") as ps_pool,
        ):
            xin = mp.tile([NPART, HW], f16, tag="xin")
            p9 = mp.tile([NK, LSP], f16, tag="p9")
            yt = mp.tile([NOUT, HHW], f16, tag="yt")
            ones_t = mp.tile([NPART, BPC], f16, tag="ones_t")
            wb_t = mp.tile([NK, NOUT], f16, tag="wb")

            p9t = p9.tensor

            csb = [
                ps_pool.tile([BPC, 512], f32, tag=f"cs{i}", name=f"cs{i}")
                for i in range(4)
            ]
            cvb = [
                ps_pool.tile([NOUT, 512], f32, tag=f"cv{i}", name=f"cv{i}")
                for i in range(4)
            ]

            # input first on the SP (HWDGE) ring, which alone sustains
            # ~360GB/s; fine-grained early pieces so cs chunks start with
            # minimal piece-boundary (completion-semaphore) quantization
            # input pieces column-interleaved across the SP (HWDGE) and
            # gpsimd (SWDGE) rings: consecutive cs chunks alternate queue
            # dependency, so per-queue generation/transfer serialization
            # stops pacing cs (Act stays pure drains)
            sp_pieces = [(0, 512), (1536, 2048), (5632, 2048), (9728, 2048), (13824, 2560)]
            gp_pieces = [(512, 1024), (3584, 2048), (7680, 2048), (11776, 2048)]
            for o, n in sp_pieces:
                nc.sync.dma_start(out=xin[:, o : o + n], in_=xh.ap()[:, o : o + n])
            for o, n in gp_pieces:
                nc.gpsimd.dma_start(out=xin[:, o : o + n], in_=xh.ap()[:, o : o + n])

            # weights after the input pieces on SP (needed only by conv)
            nc.sync.dma_start(out=wb_t[:, :], in_=wbh.ap()[:, :])

            # ones indicator [128, 2]: col b is 1 for partitions of batch b
            nc.vector.memset(ones_t[0:CIN, 0:1], 1.0)
            nc.vector.memset(ones_t[0:CIN, 1:2], 0.0)
            nc.vector.memset(ones_t[CIN:NPART, 0:1], 0.0)
            nc.vector.memset(ones_t[CIN:NPART, 1:2], 1.0)

            # staging zero borders in P9 partitions {0, 1}:
            # row -1, row 128 + tail, and (right col, next left col) pairs
            nc.vector.memset(
                AP(tensor=p9t, offset=0, ap=[[LSP, BPC], [1, WROW]]), 0.0
            )
            nc.vector.memset(
                AP(
                    tensor=p9t,
                    offset=(H + 1) * WROW,
                    ap=[[LSP, BPC], [1, LSP - (H + 1) * WROW]],
                ),
                0.0,
            )
            nc.vector.memset(
                AP(
                    tensor=p9t,
                    offset=WROW - 1,
                    ap=[[LSP, BPC], [WROW, H + 1], [1, 2]],
                ),
                0.0,
            )

            # PE p-state warm-up: 512-row garbage matmuls (moving = yt,
            # which nothing has written yet; out = csb[3], first really
            # produced by cs chunk 3) keep PE continuously busy from ~8us
            # until the input stream arrives, so the 2.4GHz ramp completes
            # before the real stream
            for _ in range(NWARM):
                nc.tensor.matmul(
                    csb[3][:, :],
                    ones_t[:, :],
                    yt[:, 0:512],
                    start=True,
                    stop=True,
                )

            copy_engines = [nc.vector, nc.scalar]

            def ecopy(idx, dst, src):
                eng = copy_engines[idx % 2]
                if eng is nc.vector:
                    eng.tensor_copy(dst, src)
                else:
                    eng.copy(dst, src)

            def emit_cs(k):
                # ones-matmul of 512 cols (4 rows) -> [2, 512]; 1 copy
                ps = csb[k % 4]
                pst = ps.tensor
                nc.tensor.matmul(
                    ps[:, :],
                    ones_t[:, :],
                    xin[:, 512 * k : 512 * k + 512],
                    start=True,
                    stop=True,
                )
                dst = AP(
                    tensor=p9t,
                    offset=(4 * k + 1) * WROW + 1,
                    ap=[[LSP, BPC], [WROW, 4], [1, W]],
                )
                src = AP(
                    tensor=pst, offset=0, ap=[[512, BPC], [W, 4], [1, W]]
                )
                ecopy(k, dst, src)

            def shifted_reads(src_t, src_pitch, r0, r1, eng):
                # rebuild p9 parts 2..17 for dst positions [r0, r1) from a
                # staging image at src_t (partition pitch src_pitch):
                # 3 DMAs, one per row-shift jj; jj=0 skips ip=0 (= staging)
                ln = r1 - r0
                eng.dma_start(
                    out=AP(
                        tensor=p9t,
                        offset=2 * LSP + r0,
                        ap=[[LSP, 4], [1, ln]],
                    ),
                    in_=AP(
                        tensor=src_t,
                        offset=r0 + 1,
                        ap=[[1, 2], [src_pitch, BPC], [1, ln]],
                    ),
                )
                for jj in (1, 2):
                    eng.dma_start(
                        out=AP(
                            tensor=p9t,
                            offset=6 * jj * LSP + r0,
                            ap=[[LSP, 6], [1, ln]],
                        ),
                        in_=AP(
                            tensor=src_t,
                            offset=r0 + WROW * jj,
                            ap=[[1, 3], [src_pitch, BPC], [1, ln]],
                        ),
                    )

            def emit_bounce(g):
                # staging segment -> HBM scratch, then the shifted reads,
                # all on the gpsimd/SWDGE ring (it is free once its input
                # pieces have gone out)
                o0, o1 = PSEG[g], PSEG[g + 1]
                nc.gpsimd.dma_start(
                    out=scratch.ap()[:, o0:o1],
                    in_=AP(tensor=p9t, offset=o0, ap=[[LSP, BPC], [1, o1 - o0]]),
                )
                shifted_reads(
                    scratch.ap().tensor, LSP, RSEG[g], RSEG[g + 1], nc.gpsimd
                )

            def emit_conv(j):
                cv = cvb[j % 4]
                nn = 512 if j < NCV - 1 else 256
                nc.tensor.matmul(
                    cv[:, :nn],
                    wb_t[:, :],
                    p9[:, 512 * j : 512 * j + nn],
                    start=True,
                    stop=True,
                )
                ecopy(j, yt[:, 512 * j : 512 * j + nn], cv[:, :nn])

            # out pieces: 7x2080 + 2x1040; piece q ready after conv chunk
            OUT_PIECES = [(2080 * q, 2080) for q in range(7)] + [
                (14560, 1040),
                (15600, 1040),
            ]
            out_after = {4: 0, 8: 1, 12: 2, 16: 3, 20: 4, 24: 5, 28: 6, 30: 7, 32: 8}

            def emit_out(q):
                o, n = OUT_PIECES[q]
                nc.sync.dma_start(
                    out=y.ap()[:, o : o + n], in_=yt[:, o : o + n]
                )

            def emit_conv_full(j):
                emit_conv(j)
                if j in out_after:
                    emit_out(out_after[j])

            for k in range(32):
                emit_cs(k)
                if k in GCH:
                    emit_bounce(GCH.index(k))
            for j in range(NCV):
                emit_conv_full(j)
            if dump:
                nc.sync.dma_start(out=p9_d.ap()[:, :], in_=p9[:, 0:HHW])

    nc.compile()
    return nc


def _host_prep(x, weight):
    wsum = weight.sum(axis=1)  # [COUT, 3, 3]
    wb = np.zeros((NK, NOUT), np.float32)
    for b in range(BPC):
        for jj in range(3):
            for ip in range(3):
                wb[6 * jj + 2 * ip + b, b * COUT : (b + 1) * COUT] = wsum[
                    :, 2 - jj, 2 - ip
                ]
    wb = wb.astype(np.float16)

    in_maps = []
    for r in range(N_CORES):
        xhr = np.ascontiguousarray(
            x[r * BPC : (r + 1) * BPC].reshape(NPART, HW)
        ).astype(np.float16)
        in_maps.append({"xh": xhr, "wb": wb})
    return in_maps


def kernel(x, weight, bias):
    from concourse.bass_utils import run_bass_kernel_spmd

    x = np.asarray(x)
    weight = np.asarray(weight)
    bias = np.asarray(bias)
    nc = _build()
    in_maps = _host_prep(x, weight)
    res = run_bass_kernel_spmd(nc, in_maps, core_ids=list(range(N_CORES)))
    out = np.concatenate(
        [
            np.asarray(res.results[r]["y"])
            .astype(np.float32)
            .reshape(BPC, COUT, H, WROW)[:, :, :, :W]
            for r in range(N_CORES)
        ],
        axis=0,
    )
    return out + bias.astype(np.float32)[None, :, None, None]


# revision 28
# speedup vs baseline: 1.0970x; 1.0016x over previous
"""FFTConv2d kernel for trn2, 8 NeuronCores.

Math: reference einsum 'bchw,oihw->bohw' factorizes:
  Y[b,o] = conv_same(sum_c x[b,c], flip(sum_i w[o,i])) + bias[o]
i.e. a single-channel 3x3 "same" convolution per (b,o) pair.
bias is added on the host (it is a [64] vector on a [16,64,128,128]
output; negligible), so no ones/bias row rides the matmul.

Per core (2 batches), all SBUF data fp16 (PSUM accum fp32):
  1. xin [128 (b,c), 16384] <- x fp16, 9 HBM DMA pieces (SP ring),
     emitted first; cs chunks chase the pieces (input ~11.7us is the
     cs-phase wall at ~360GB/s).
  2. A dozen tiny warm-up matmuls keep PE busy early so the p-state
     ramp (2.4GHz after ~3us continuous) completes before the real
     stream starts.
  3. Channel-sum: ones-indicator matmul pairs -> PSUM [2, 512];
     FD=512 copies (DVE/Act alternating) drain 4 image rows into the
     padded staging = P9 partitions {0,1} (row stride 130, zero
     borders memset once).
  4. P9 [18, 16902], partition p=6jj+2ip+b holds staging shifted by
     130jj+ip; p=0,1 IS the staging; shifts built via a DRAM bounce
     (SBUF->SBUF DMA is ~5x slower per byte than HBM paths): 5
     staging segments written to an Internal HBM scratch as they
     drain (gpsimd/SWDGE ring), then per-segment shifted reads (3
     DMAs, one per row-shift jj) rebuild the 16 shifted partitions.
     The last sliver skips the bounce (direct SBUF->SBUF, one hop)
     to shorten the post-cs critical path.
  5. Conv: 33 flat 512-col chunks; K=18 fp16 matmuls into a 4-deep
     PSUM rotation; FD=512 copies -> yt. Conv chunks are emitted
     interleaved into the cs stream once their staging segment is
     expected, soaking up PE idle while cs waits on input DMA.
  6. yt [128, 16640] -> HBM in 9 pieces on the SP ring (7x2080 +
     2x1040 so the final piece is small); host strips the 2 junk
     cols per 130-wide row, upconverts to fp32, adds bias.
"""

import os
import sys
from functools import lru_cache

import numpy as np

for _p in ("/opt/trn_rl_repo", "/root/.axon_site/_ro/trn_rl_repo"):
    if os.path.isdir(_p) and _p not in sys.path:
        sys.path.insert(0, _p)

B, CIN, COUT, H, W = 16, 64, 64, 128, 128
N_CORES = 8
BPC = B // N_CORES  # 2
NPART = BPC * CIN  # 128
NOUT = BPC * COUT  # 128
WROW = W + 2  # 130
HW = H * W  # 16384
HHW = H * WROW  # 16640 (130-wide output rows)
LSP = (H + 2) * WROW + 2  # 16902 (padded staging length)
NK = BPC * 9  # 18
NCV = 33  # conv chunks: 32x512 + 1x256
NWARM = 0

# staging segment g is in scratch after cs chunk GCH[g] drains
# (chunk c covers staging positions < (4c+5)*130 + 1 incl. borders);
# shifted read g rebuilds p9 dst positions [RSEG[g], RSEG[g+1])
# (needs src to dst_end + 262). Final sliver [RSEG[5], 16640) goes
# direct SBUF->SBUF after the last drain.
GCH = [9, 19, 31]
PSEG = [0] + [(4 * c + 5) * WROW + 1 for c in GCH[:-1]] + [LSP]
RSEG = [0] + [PSEG[g + 1] - 262 for g in range(2)] + [HHW]
# conv chunks unlocked by read g (chunk j needs dst < 512j+512):
#   g0: 0-8, g1: 9-19, g2: 20-32
# strict cs -> conv phases: every read group lands well before the
# in-order conv train reaches its chunks (interleaving conv into the
# cs stream measurably backfires: one late read stalls PE for all
# downstream work)


@lru_cache(maxsize=1)
def _build():
    import concourse.bacc as bacc
    import concourse.mybir as mybir
    import concourse.tile as tile
    from concourse.ap import AP

    f32 = mybir.dt.float32
    f16 = mybir.dt.float16

    nc = bacc.Bacc("TRN2", target_bir_lowering=False, debug=False, num_devices=N_CORES)

    xh = nc.dram_tensor("xh", [NPART, HW], f16, kind="ExternalInput")
    wbh = nc.dram_tensor("wb", [NK, NOUT], f16, kind="ExternalInput")
    y = nc.dram_tensor("y", [NOUT, HHW], f16, kind="ExternalOutput")
    scratch = nc.dram_tensor("xs_scratch", [BPC, LSP], f16, kind="Internal")
    dump = os.environ.get("KDUMP")
    if dump:
        p9_d = nc.dram_tensor("p9_d", [NK, HHW], f16, kind="ExternalOutput")

    with tile.TileContext(nc) as tc:
        with (
            tc.tile_pool(name="main", bufs=1) as mp,
            tc.tile_pool(name="ps", bufs=1, space="PSUM") as ps_pool,
        ):
            xin = mp.tile([NPART, HW], f16, tag="xin")
            p9 = mp.tile([NK, LSP], f16, tag="p9")
            yt = mp.tile([NOUT, HHW], f16, tag="yt")
            ones_t = mp.tile([NPART, BPC], f16, tag="ones_t")
            wb_t = mp.tile([NK, NOUT], f16, tag="wb")

            p9t = p9.tensor

            csb = [
                ps_pool.tile([BPC, 512], f32, tag=f"cs{i}", name=f"cs{i}")
                for i in range(4)
            ]
            cvb = [
                ps_pool.tile([NOUT, 512], f32, tag=f"cv{i}", name=f"cv{i}")
                for i in range(4)
            ]

            # input first on the SP (HWDGE) ring, which alone sustains
            # ~360GB/s; fine-grained early pieces so cs chunks start with
            # minimal piece-boundary (completion-semaphore) quantization
            # input pieces column-interleaved across the SP (HWDGE) and
            # gpsimd (SWDGE) rings: consecutive cs chunks alternate queue
            # dependency, so per-queue generation/transfer serialization
            # stops pacing cs (Act stays pure drains)
            sp_pieces = [(0, 1024), (2048, 2048), (6144, 2048), (10240, 2048), (14336, 2048)]
            gp_pieces = [(1024, 1024), (4096, 2048), (8192, 2048), (12288, 2048)]
            for o, n in sp_pieces:
                nc.sync.dma_start(out=xin[:, o : o + n], in_=xh.ap()[:, o : o + n])
            for o, n in gp_pieces:
                nc.gpsimd.dma_start(out=xin[:, o : o + n], in_=xh.ap()[:, o : o + n])

            # weights after the input pieces on SP (needed only by conv)
            nc.sync.dma_start(out=wb_t[:, :], in_=wbh.ap()[:, :])

            # ones indicator [128, 2]: col b is 1 for partitions of batch b
            nc.vector.memset(ones_t[0:CIN, 0:1], 1.0)
            nc.vector.memset(ones_t[0:CIN, 1:2], 0.0)
            nc.vector.memset(ones_t[CIN:NPART, 0:1], 0.0)
            nc.vector.memset(ones_t[CIN:NPART, 1:2], 1.0)

            # staging zero borders in P9 partitions {0, 1}:
            # row -1, row 128 + tail, and (right col, next left col) pairs
            nc.vector.memset(
                AP(tensor=p9t, offset=0, ap=[[LSP, BPC], [1, WROW]]), 0.0
            )
            nc.vector.memset(
                AP(
                    tensor=p9t,
                    offset=(H + 1) * WROW,
                    ap=[[LSP, BPC], [1, LSP - (H + 1) * WROW]],
                ),
                0.0,
            )
            nc.vector.memset(
                AP(
                    tensor=p9t,
                    offset=WROW - 1,
                    ap=[[LSP, BPC], [WROW, H + 1], [1, 2]],
                ),
                0.0,
            )

            # PE p-state warm-up: 512-row garbage matmuls (moving = yt,
            # which nothing has written yet; out = csb[3], first really
            # produced by cs chunk 3) keep PE continuously busy from ~8us
            # until the input stream arrives, so the 2.4GHz ramp completes
            # before the real stream
            for _ in range(NWARM):
                nc.tensor.matmul(
                    csb[3][:, :],
                    ones_t[:, :],
                    yt[:, 0:512],
                    start=True,
                    stop=True,
                )

            copy_engines = [nc.vector, nc.scalar]

            def ecopy(idx, dst, src):
                eng = copy_engines[idx % 2]
                if eng is nc.vector:
                    eng.tensor_copy(dst, src)
                else:
                    eng.copy(dst, src)

            def emit_cs(k):
                # ones-matmul of 512 cols (4 rows) -> [2, 512]; 1 copy
                ps = csb[k % 4]
                pst = ps.tensor
                nc.tensor.matmul(
                    ps[:, :],
                    ones_t[:, :],
                    xin[:, 512 * k : 512 * k + 512],
                    start=True,
                    stop=True,
                )
                dst = AP(
                    tensor=p9t,
                    offset=(4 * k + 1) * WROW + 1,
                    ap=[[LSP, BPC], [WROW, 4], [1, W]],
                )
                src = AP(
                    tensor=pst, offset=0, ap=[[512, BPC], [W, 4], [1, W]]
                )
                ecopy(k, dst, src)

            def shifted_reads(src_t, src_pitch, r0, r1, eng):
                # rebuild p9 parts 2..17 for dst positions [r0, r1) from a
                # staging image at src_t (partition pitch src_pitch):
                # 3 DMAs, one per row-shift jj; jj=0 skips ip=0 (= staging)
                ln = r1 - r0
                eng.dma_start(
                    out=AP(
                        tensor=p9t,
                        offset=2 * LSP + r0,
                        ap=[[LSP, 4], [1, ln]],
                    ),
                    in_=AP(
                        tensor=src_t,
                        offset=r0 + 1,
                        ap=[[1, 2], [src_pitch, BPC], [1, ln]],
                    ),
                )
                for jj in (1, 2):
                    eng.dma_start(
                        out=AP(
                            tensor=p9t,
                            offset=6 * jj * LSP + r0,
                            ap=[[LSP, 6], [1, ln]],
                        ),
                        in_=AP(
                            tensor=src_t,
                            offset=r0 + WROW * jj,
                            ap=[[1, 3], [src_pitch, BPC], [1, ln]],
                        ),
                    )

            def emit_bounce(g):
                # staging segment -> HBM scratch, then the shifted reads,
                # all on the gpsimd/SWDGE ring (it is free once its input
                # pieces have gone out)
                o0, o1 = PSEG[g], PSEG[g + 1]
                nc.gpsimd.dma_start(
                    out=scratch.ap()[:, o0:o1],
                    in_=AP(tensor=p9t, offset=o0, ap=[[LSP, BPC], [1, o1 - o0]]),
                )
                shifted_reads(
                    scratch.ap().tensor, LSP, RSEG[g], RSEG[g + 1], nc.gpsimd
                )

            def emit_conv(j):
                cv = cvb[j % 4]
                nn = 512 if j < NCV - 1 else 256
                nc.tensor.matmul(
                    cv[:, :nn],
                    wb_t[:, :],
                    p9[:, 512 * j : 512 * j + nn],
                    start=True,
                    stop=True,
                )
                ecopy(j, yt[:, 512 * j : 512 * j + nn], cv[:, :nn])

            # out pieces: 7x2080 + 2x1040; piece q ready after conv chunk
            OUT_PIECES = [(2080 * q, 2080) for q in range(7)] + [
                (14560, 1040),
                (15600, 1040),
            ]
            out_after = {4: 0, 8: 1, 12: 2, 16: 3, 20: 4, 24: 5, 28: 6, 30: 7, 32: 8}

            def emit_out(q):
                o, n = OUT_PIECES[q]
                nc.sync.dma_start(
                    out=y.ap()[:, o : o + n], in_=yt[:, o : o + n]
                )

            def emit_conv_full(j):
                emit_conv(j)
                if j in out_after:
                    emit_out(out_after[j])

            for k in range(32):
                emit_cs(k)
                if k in GCH:
                    emit_bounce(GCH.index(k))
            for j in range(NCV):
                emit_conv_full(j)
            if dump:
                nc.sync.dma_start(out=p9_d.ap()[:, :], in_=p9[:, 0:HHW])

    nc.compile()
    return nc


def _host_prep(x, weight):
    wsum = weight.sum(axis=1)  # [COUT, 3, 3]
    wb = np.zeros((NK, NOUT), np.float32)
    for b in range(BPC):
        for jj in range(3):
            for ip in range(3):
                wb[6 * jj + 2 * ip + b, b * COUT : (b + 1) * COUT] = wsum[
                    :, 2 - jj, 2 - ip
                ]
    wb = wb.astype(np.float16)

    in_maps = []
    for r in range(N_CORES):
        xhr = np.ascontiguousarray(
            x[r * BPC : (r + 1) * BPC].reshape(NPART, HW)
        ).astype(np.float16)
        in_maps.append({"xh": xhr, "wb": wb})
    return in_maps


def kernel(x, weight, bias):
    from concourse.bass_utils import run_bass_kernel_spmd

    x = np.asarray(x)
    weight = np.asarray(weight)
    bias = np.asarray(bias)
    nc = _build()
    in_maps = _host_prep(x, weight)
    res = run_bass_kernel_spmd(nc, in_maps, core_ids=list(range(N_CORES)))
    out = np.concatenate(
        [
            np.asarray(res.results[r]["y"])
            .astype(np.float32)
            .reshape(BPC, COUT, H, WROW)[:, :, :, :W]
            for r in range(N_CORES)
        ],
        axis=0,
    )
    return out + bias.astype(np.float32)[None, :, None, None]


# revision 29
# speedup vs baseline: 1.1220x; 1.0228x over previous
"""FFTConv2d kernel for trn2, 8 NeuronCores.

Math: reference einsum 'bchw,oihw->bohw' factorizes:
  Y[b,o] = conv_same(sum_c x[b,c], flip(sum_i w[o,i])) + bias[o]
i.e. a single-channel 3x3 "same" convolution per (b,o) pair.
bias is added on the host (it is a [64] vector on a [16,64,128,128]
output; negligible), so no ones/bias row rides the matmul.

Per core (2 batches), all SBUF data fp16 (PSUM accum fp32):
  1. xin [128 (b,c), 16384] <- x fp16, 9 HBM DMA pieces (SP ring),
     emitted first; cs chunks chase the pieces (input ~11.7us is the
     cs-phase wall at ~360GB/s).
  2. A dozen tiny warm-up matmuls keep PE busy early so the p-state
     ramp (2.4GHz after ~3us continuous) completes before the real
     stream starts.
  3. Channel-sum: ones-indicator matmul pairs -> PSUM [2, 512];
     FD=512 copies (DVE/Act alternating) drain 4 image rows into the
     padded staging = P9 partitions {0,1} (row stride 130, zero
     borders memset once).
  4. P9 [18, 16902], partition p=6jj+2ip+b holds staging shifted by
     130jj+ip; p=0,1 IS the staging; shifts built via a DRAM bounce
     (SBUF->SBUF DMA is ~5x slower per byte than HBM paths): 5
     staging segments written to an Internal HBM scratch as they
     drain (gpsimd/SWDGE ring), then per-segment shifted reads (3
     DMAs, one per row-shift jj) rebuild the 16 shifted partitions.
     The last sliver skips the bounce (direct SBUF->SBUF, one hop)
     to shorten the post-cs critical path.
  5. Conv: 33 flat 512-col chunks; K=18 fp16 matmuls into a 4-deep
     PSUM rotation; FD=512 copies -> yt. Conv chunks are emitted
     interleaved into the cs stream once their staging segment is
     expected, soaking up PE idle while cs waits on input DMA.
  6. yt [128, 16640] -> HBM in 9 pieces on the SP ring (7x2080 +
     2x1040 so the final piece is small); host strips the 2 junk
     cols per 130-wide row, upconverts to fp32, adds bias.
"""

import os
import sys
from functools import lru_cache

import numpy as np

for _p in ("/opt/trn_rl_repo", "/root/.axon_site/_ro/trn_rl_repo"):
    if os.path.isdir(_p) and _p not in sys.path:
        sys.path.insert(0, _p)

B, CIN, COUT, H, W = 16, 64, 64, 128, 128
N_CORES = 8
BPC = B // N_CORES  # 2
NPART = BPC * CIN  # 128
NOUT = BPC * COUT  # 128
WROW = W + 2  # 130
HW = H * W  # 16384
HHW = H * WROW  # 16640 (130-wide output rows)
LSP = (H + 2) * WROW + 2  # 16902 (padded staging length)
NK = BPC * 9  # 18
NCV = 33  # conv chunks: 32x512 + 1x256
NWARM = 0

# staging segment g is in scratch after cs chunk GCH[g] drains
# (chunk c covers staging positions < (4c+5)*130 + 1 incl. borders);
# shifted read g rebuilds p9 dst positions [RSEG[g], RSEG[g+1])
# (needs src to dst_end + 262). Final sliver [RSEG[5], 16640) goes
# direct SBUF->SBUF after the last drain.
GCH = [7, 15, 23, 31]
PSEG = [0] + [(4 * c + 5) * WROW + 1 for c in GCH[:-1]] + [LSP]
RSEG = [0] + [PSEG[g + 1] - 262 for g in range(3)] + [HHW]
# conv chunks unlocked by read g (chunk j needs dst < 512j+512):
#   g0: 0-6, g1: 7-14, g2: 15-23, g3: 24-32
# strict cs -> conv phases: every read group lands well before the
# in-order conv train reaches its chunks (interleaving conv into the
# cs stream measurably backfires: one late read stalls PE for all
# downstream work)


@lru_cache(maxsize=1)
def _build():
    import concourse.bacc as bacc
    import concourse.mybir as mybir
    import concourse.tile as tile
    from concourse.ap import AP

    f32 = mybir.dt.float32
    f16 = mybir.dt.float16

    nc = bacc.Bacc("TRN2", target_bir_lowering=False, debug=False, num_devices=N_CORES)

    xh = nc.dram_tensor("xh", [NPART, HW], f16, kind="ExternalInput")
    wbh = nc.dram_tensor("wb", [NK, NOUT], f16, kind="ExternalInput")
    y = nc.dram_tensor("y", [NOUT, HHW], f16, kind="ExternalOutput")
    scratch = nc.dram_tensor("xs_scratch", [BPC, LSP], f16, kind="Internal")
    dump = os.environ.get("KDUMP")
    if dump:
        p9_d = nc.dram_tensor("p9_d", [NK, HHW], f16, kind="ExternalOutput")

    with tile.TileContext(nc) as tc:
        with (
            tc.tile_pool(name="main", bufs=1) as mp,
            tc.tile_pool(name="ps", bufs=1, space="PSUM") as ps_pool,
        ):
            xin = mp.tile([NPART, HW], f16, tag="xin")
            p9 = mp.tile([NK, LSP], f16, tag="p9")
            yt = mp.tile([NOUT, HHW], f16, tag="yt")
            ones_t = mp.tile([NPART, BPC], f16, tag="ones_t")
            wb_t = mp.tile([NK, NOUT], f16, tag="wb")

            p9t = p9.tensor

            csb = [
                ps_pool.tile([BPC, 512], f32, tag=f"cs{i}", name=f"cs{i}")
                for i in range(4)
            ]
            cvb = [
                ps_pool.tile([NOUT, 512], f32, tag=f"cv{i}", name=f"cv{i}")
                for i in range(4)
            ]

            # input first on the SP (HWDGE) ring, which alone sustains
            # ~360GB/s; fine-grained early pieces so cs chunks start with
            # minimal piece-boundary (completion-semaphore) quantization
            # input pieces column-interleaved across the SP (HWDGE) and
            # gpsimd (SWDGE) rings: consecutive cs chunks alternate queue
            # dependency, so per-queue generation/transfer serialization
            # stops pacing cs (Act stays pure drains)
            # weights first on the (otherwise idle-at-start) gpsimd ring
            nc.gpsimd.dma_start(out=wb_t[:, :], in_=wbh.ap()[:, :])
            pieces = [(0, 1024), (1024, 1024)] + [
                (2048 * q, 2048) for q in range(1, 8)
            ]
            for o, n in pieces:
                nc.sync.dma_start(out=xin[:, o : o + n], in_=xh.ap()[:, o : o + n])

            # ones indicator [128, 2]: col b is 1 for partitions of batch b
            nc.vector.memset(ones_t[0:CIN, 0:1], 1.0)
            nc.vector.memset(ones_t[0:CIN, 1:2], 0.0)
            nc.vector.memset(ones_t[CIN:NPART, 0:1], 0.0)
            nc.vector.memset(ones_t[CIN:NPART, 1:2], 1.0)

            # staging zero borders in P9 partitions {0, 1}:
            # row -1, row 128 + tail, and (right col, next left col) pairs
            nc.vector.memset(
                AP(tensor=p9t, offset=0, ap=[[LSP, BPC], [1, WROW]]), 0.0
            )
            nc.vector.memset(
                AP(
                    tensor=p9t,
                    offset=(H + 1) * WROW,
                    ap=[[LSP, BPC], [1, LSP - (H + 1) * WROW]],
                ),
                0.0,
            )
            nc.vector.memset(
                AP(
                    tensor=p9t,
                    offset=WROW - 1,
                    ap=[[LSP, BPC], [WROW, H + 1], [1, 2]],
                ),
                0.0,
            )

            # PE p-state warm-up: 512-row garbage matmuls (moving = yt,
            # which nothing has written yet; out = csb[3], first really
            # produced by cs chunk 3) keep PE continuously busy from ~8us
            # until the input stream arrives, so the 2.4GHz ramp completes
            # before the real stream
            for _ in range(NWARM):
                nc.tensor.matmul(
                    csb[3][:, :],
                    ones_t[:, :],
                    yt[:, 0:512],
                    start=True,
                    stop=True,
                )

            copy_engines = [nc.vector, nc.scalar]

            def ecopy(idx, dst, src):
                eng = copy_engines[idx % 2]
                if eng is nc.vector:
                    eng.tensor_copy(dst, src)
                else:
                    eng.copy(dst, src)

            def emit_cs(k):
                # ones-matmul of 512 cols (4 rows) -> [2, 512]; 1 copy
                ps = csb[k % 4]
                pst = ps.tensor
                nc.tensor.matmul(
                    ps[:, :],
                    ones_t[:, :],
                    xin[:, 512 * k : 512 * k + 512],
                    start=True,
                    stop=True,
                )
                dst = AP(
                    tensor=p9t,
                    offset=(4 * k + 1) * WROW + 1,
                    ap=[[LSP, BPC], [WROW, 4], [1, W]],
                )
                src = AP(
                    tensor=pst, offset=0, ap=[[512, BPC], [W, 4], [1, W]]
                )
                ecopy(k, dst, src)

            def shifted_reads(src_t, src_pitch, r0, r1, eng):
                # rebuild p9 parts 2..17 for dst positions [r0, r1) from a
                # staging image at src_t (partition pitch src_pitch):
                # 3 DMAs, one per row-shift jj; jj=0 skips ip=0 (= staging)
                ln = r1 - r0
                eng.dma_start(
                    out=AP(
                        tensor=p9t,
                        offset=2 * LSP + r0,
                        ap=[[LSP, 4], [1, ln]],
                    ),
                    in_=AP(
                        tensor=src_t,
                        offset=r0 + 1,
                        ap=[[1, 2], [src_pitch, BPC], [1, ln]],
                    ),
                )
                for jj in (1, 2):
                    eng.dma_start(
                        out=AP(
                            tensor=p9t,
                            offset=6 * jj * LSP + r0,
                            ap=[[LSP, 6], [1, ln]],
                        ),
                        in_=AP(
                            tensor=src_t,
                            offset=r0 + WROW * jj,
                            ap=[[1, 3], [src_pitch, BPC], [1, ln]],
                        ),
                    )

            def emit_bounce(g):
                # staging segment -> HBM scratch, then the shifted reads,
                # all on the gpsimd/SWDGE ring (it is free once its input
                # pieces have gone out)
                o0, o1 = PSEG[g], PSEG[g + 1]
                nc.gpsimd.dma_start(
                    out=scratch.ap()[:, o0:o1],
                    in_=AP(tensor=p9t, offset=o0, ap=[[LSP, BPC], [1, o1 - o0]]),
                )
                shifted_reads(
                    scratch.ap().tensor, LSP, RSEG[g], RSEG[g + 1], nc.gpsimd
                )

            def emit_conv(j):
                cv = cvb[j % 4]
                nn = 512 if j < NCV - 1 else 256
                nc.tensor.matmul(
                    cv[:, :nn],
                    wb_t[:, :],
                    p9[:, 512 * j : 512 * j + nn],
                    start=True,
                    stop=True,
                )
                ecopy(j, yt[:, 512 * j : 512 * j + nn], cv[:, :nn])

            # out pieces: 7x2080 + 2x1040; piece q ready after conv chunk
            OUT_PIECES = [(2080 * q, 2080) for q in range(7)] + [
                (14560, 1040),
                (15600, 1040),
            ]
            out_after = {4: 0, 8: 1, 12: 2, 16: 3, 20: 4, 24: 5, 28: 6, 30: 7, 32: 8}

            def emit_out(q):
                o, n = OUT_PIECES[q]
                nc.sync.dma_start(
                    out=y.ap()[:, o : o + n], in_=yt[:, o : o + n]
                )

            def emit_conv_full(j):
                emit_conv(j)
                if j in out_after:
                    emit_out(out_after[j])

            for k in range(32):
                emit_cs(k)
                if k in GCH:
                    emit_bounce(GCH.index(k))
            for j in range(NCV):
                emit_conv_full(j)
            if dump:
                nc.sync.dma_start(out=p9_d.ap()[:, :], in_=p9[:, 0:HHW])

    nc.compile()
    return nc


def _host_prep(x, weight):
    wsum = weight.sum(axis=1)  # [COUT, 3, 3]
    wb = np.zeros((NK, NOUT), np.float32)
    for b in range(BPC):
        for jj in range(3):
            for ip in range(3):
                wb[6 * jj + 2 * ip + b, b * COUT : (b + 1) * COUT] = wsum[
                    :, 2 - jj, 2 - ip
                ]
    wb = wb.astype(np.float16)

    in_maps = []
    for r in range(N_CORES):
        xhr = np.ascontiguousarray(
            x[r * BPC : (r + 1) * BPC].reshape(NPART, HW)
        ).astype(np.float16)
        in_maps.append({"xh": xhr, "wb": wb})
    return in_maps


def kernel(x, weight, bias):
    from concourse.bass_utils import run_bass_kernel_spmd

    x = np.asarray(x)
    weight = np.asarray(weight)
    bias = np.asarray(bias)
    nc = _build()
    in_maps = _host_prep(x, weight)
    res = run_bass_kernel_spmd(nc, in_maps, core_ids=list(range(N_CORES)))
    out = np.concatenate(
        [
            np.asarray(res.results[r]["y"])
            .astype(np.float32)
            .reshape(BPC, COUT, H, WROW)[:, :, :, :W]
            for r in range(N_CORES)
        ],
        axis=0,
    )
    return out + bias.astype(np.float32)[None, :, None, None]


# revision 30
# speedup vs baseline: 1.1365x; 1.0129x over previous
"""FFTConv2d kernel for trn2, 8 NeuronCores.

Math: reference einsum 'bchw,oihw->bohw' factorizes:
  Y[b,o] = conv_same(sum_c x[b,c], flip(sum_i w[o,i])) + bias[o]
i.e. a single-channel 3x3 "same" convolution per (b,o) pair.
bias is added on the host (it is a [64] vector on a [16,64,128,128]
output; negligible), so no ones/bias row rides the matmul.

Per core (2 batches), all SBUF data fp16 (PSUM accum fp32):
  1. xin [128 (b,c), 16384] <- x fp16, 9 HBM DMA pieces (SP ring),
     emitted first; cs chunks chase the pieces (input ~11.7us is the
     cs-phase wall at ~360GB/s).
  2. A dozen tiny warm-up matmuls keep PE busy early so the p-state
     ramp (2.4GHz after ~3us continuous) completes before the real
     stream starts.
  3. Channel-sum: ones-indicator matmul pairs -> PSUM [2, 512];
     FD=512 copies (DVE/Act alternating) drain 4 image rows into the
     padded staging = P9 partitions {0,1} (row stride 130, zero
     borders memset once).
  4. P9 [18, 16902], partition p=6jj+2ip+b holds staging shifted by
     130jj+ip; p=0,1 IS the staging; shifts built via a DRAM bounce
     (SBUF->SBUF DMA is ~5x slower per byte than HBM paths): 5
     staging segments written to an Internal HBM scratch as they
     drain (gpsimd/SWDGE ring), then per-segment shifted reads (3
     DMAs, one per row-shift jj) rebuild the 16 shifted partitions.
     The last sliver skips the bounce (direct SBUF->SBUF, one hop)
     to shorten the post-cs critical path.
  5. Conv: 33 flat 512-col chunks; K=18 fp16 matmuls into a 4-deep
     PSUM rotation; FD=512 copies -> yt. Conv chunks are emitted
     interleaved into the cs stream once their staging segment is
     expected, soaking up PE idle while cs waits on input DMA.
  6. yt [128, 16640] -> HBM in 9 pieces on the SP ring (7x2080 +
     2x1040 so the final piece is small); host strips the 2 junk
     cols per 130-wide row, upconverts to fp32, adds bias.
"""

import os
import sys
from functools import lru_cache

import numpy as np

for _p in ("/opt/trn_rl_repo", "/root/.axon_site/_ro/trn_rl_repo"):
    if os.path.isdir(_p) and _p not in sys.path:
        sys.path.insert(0, _p)

B, CIN, COUT, H, W = 16, 64, 64, 128, 128
N_CORES = 8
BPC = B // N_CORES  # 2
NPART = BPC * CIN  # 128
NOUT = BPC * COUT  # 128
WROW = W + 2  # 130
HW = H * W  # 16384
HHW = H * WROW  # 16640 (130-wide output rows)
LSP = (H + 2) * WROW + 2  # 16902 (padded staging length)
NK = BPC * 9  # 18
NCV = 33  # conv chunks: 32x512 + 1x256
NWARM = 0

# staging segment g is in scratch after cs chunk GCH[g] drains
# (chunk c covers staging positions < (4c+5)*130 + 1 incl. borders);
# shifted read g rebuilds p9 dst positions [RSEG[g], RSEG[g+1])
# (needs src to dst_end + 262). Final sliver [RSEG[5], 16640) goes
# direct SBUF->SBUF after the last drain.
GCH = [5, 13, 21, 31]
PSEG = [0] + [(4 * c + 5) * WROW + 1 for c in GCH[:-1]] + [LSP]
RSEG = [0] + [PSEG[g + 1] - 262 for g in range(3)] + [HHW]
# conv chunks unlocked by read g (chunk j needs dst < 512j+512):
#   g0: 0-4, g1: 5-12, g2: 13-21, g3: 22-32
# strict cs -> conv phases: every read group lands well before the
# in-order conv train reaches its chunks (interleaving conv into the
# cs stream measurably backfires: one late read stalls PE for all
# downstream work)


@lru_cache(maxsize=1)
def _build():
    import concourse.bacc as bacc
    import concourse.mybir as mybir
    import concourse.tile as tile
    from concourse.ap import AP

    f32 = mybir.dt.float32
    f16 = mybir.dt.float16

    nc = bacc.Bacc("TRN2", target_bir_lowering=False, debug=False, num_devices=N_CORES)

    xh = nc.dram_tensor("xh", [NPART, HW], f16, kind="ExternalInput")
    wbh = nc.dram_tensor("wb", [NK, NOUT], f16, kind="ExternalInput")
    y = nc.dram_tensor("y", [NOUT, HHW], f16, kind="ExternalOutput")
    scratch = nc.dram_tensor("xs_scratch", [BPC, LSP], f16, kind="Internal")
    dump = os.environ.get("KDUMP")
    if dump:
        p9_d = nc.dram_tensor("p9_d", [NK, HHW], f16, kind="ExternalOutput")

    with tile.TileContext(nc) as tc:
        with (
            tc.tile_pool(name="main", bufs=1) as mp,
            tc.tile_pool(name="ps", bufs=1, space="PSUM") as ps_pool,
        ):
            xin = mp.tile([NPART, HW], f16, tag="xin")
            p9 = mp.tile([NK, LSP], f16, tag="p9")
            yt = mp.tile([NOUT, HHW], f16, tag="yt")
            ones_t = mp.tile([NPART, BPC], f16, tag="ones_t")
            wb_t = mp.tile([NK, NOUT], f16, tag="wb")

            p9t = p9.tensor

            csb = [
                ps_pool.tile([BPC, 512], f32, tag=f"cs{i}", name=f"cs{i}")
                for i in range(4)
            ]
            cvb = [
                ps_pool.tile([NOUT, 512], f32, tag=f"cv{i}", name=f"cv{i}")
                for i in range(4)
            ]

            # input first on the SP (HWDGE) ring, which alone sustains
            # ~360GB/s; fine-grained early pieces so cs chunks start with
            # minimal piece-boundary (completion-semaphore) quantization
            # input pieces column-interleaved across the SP (HWDGE) and
            # gpsimd (SWDGE) rings: consecutive cs chunks alternate queue
            # dependency, so per-queue generation/transfer serialization
            # stops pacing cs (Act stays pure drains)
            # weights first on the (otherwise idle-at-start) gpsimd ring
            nc.gpsimd.dma_start(out=wb_t[:, :], in_=wbh.ap()[:, :])
            pieces = [(0, 1024), (1024, 1024)] + [
                (2048 * q, 2048) for q in range(1, 8)
            ]
            for o, n in pieces:
                nc.sync.dma_start(out=xin[:, o : o + n], in_=xh.ap()[:, o : o + n])

            # ones indicator [128, 2]: col b is 1 for partitions of batch b
            nc.vector.memset(ones_t[0:CIN, 0:1], 1.0)
            nc.vector.memset(ones_t[0:CIN, 1:2], 0.0)
            nc.vector.memset(ones_t[CIN:NPART, 0:1], 0.0)
            nc.vector.memset(ones_t[CIN:NPART, 1:2], 1.0)

            # staging zero borders in P9 partitions {0, 1}:
            # row -1, row 128 + tail, and (right col, next left col) pairs
            nc.vector.memset(
                AP(tensor=p9t, offset=0, ap=[[LSP, BPC], [1, WROW]]), 0.0
            )
            nc.vector.memset(
                AP(
                    tensor=p9t,
                    offset=(H + 1) * WROW,
                    ap=[[LSP, BPC], [1, LSP - (H + 1) * WROW]],
                ),
                0.0,
            )
            nc.vector.memset(
                AP(
                    tensor=p9t,
                    offset=WROW - 1,
                    ap=[[LSP, BPC], [WROW, H + 1], [1, 2]],
                ),
                0.0,
            )

            # PE p-state warm-up: 512-row garbage matmuls (moving = yt,
            # which nothing has written yet; out = csb[3], first really
            # produced by cs chunk 3) keep PE continuously busy from ~8us
            # until the input stream arrives, so the 2.4GHz ramp completes
            # before the real stream
            for _ in range(NWARM):
                nc.tensor.matmul(
                    csb[3][:, :],
                    ones_t[:, :],
                    yt[:, 0:512],
                    start=True,
                    stop=True,
                )

            copy_engines = [nc.vector, nc.scalar]

            def ecopy(idx, dst, src):
                eng = copy_engines[idx % 2]
                if eng is nc.vector:
                    eng.tensor_copy(dst, src)
                else:
                    eng.copy(dst, src)

            def emit_cs(k):
                # ones-matmul of 512 cols (4 rows) -> [2, 512]; 1 copy
                ps = csb[k % 4]
                pst = ps.tensor
                nc.tensor.matmul(
                    ps[:, :],
                    ones_t[:, :],
                    xin[:, 512 * k : 512 * k + 512],
                    start=True,
                    stop=True,
                )
                dst = AP(
                    tensor=p9t,
                    offset=(4 * k + 1) * WROW + 1,
                    ap=[[LSP, BPC], [WROW, 4], [1, W]],
                )
                src = AP(
                    tensor=pst, offset=0, ap=[[512, BPC], [W, 4], [1, W]]
                )
                ecopy(k, dst, src)

            def shifted_reads(src_t, src_pitch, r0, r1, eng):
                # rebuild p9 parts 2..17 for dst positions [r0, r1) from a
                # staging image at src_t (partition pitch src_pitch):
                # 3 DMAs, one per row-shift jj; jj=0 skips ip=0 (= staging)
                ln = r1 - r0
                eng.dma_start(
                    out=AP(
                        tensor=p9t,
                        offset=2 * LSP + r0,
                        ap=[[LSP, 4], [1, ln]],
                    ),
                    in_=AP(
                        tensor=src_t,
                        offset=r0 + 1,
                        ap=[[1, 2], [src_pitch, BPC], [1, ln]],
                    ),
                )
                for jj in (1, 2):
                    eng.dma_start(
                        out=AP(
                            tensor=p9t,
                            offset=6 * jj * LSP + r0,
                            ap=[[LSP, 6], [1, ln]],
                        ),
                        in_=AP(
                            tensor=src_t,
                            offset=r0 + WROW * jj,
                            ap=[[1, 3], [src_pitch, BPC], [1, ln]],
                        ),
                    )

            def emit_bounce(g):
                # staging segment -> HBM scratch, then the shifted reads,
                # all on the gpsimd/SWDGE ring (it is free once its input
                # pieces have gone out)
                o0, o1 = PSEG[g], PSEG[g + 1]
                nc.gpsimd.dma_start(
                    out=scratch.ap()[:, o0:o1],
                    in_=AP(tensor=p9t, offset=o0, ap=[[LSP, BPC], [1, o1 - o0]]),
                )
                shifted_reads(
                    scratch.ap().tensor, LSP, RSEG[g], RSEG[g + 1], nc.gpsimd
                )

            def emit_conv(j):
                cv = cvb[j % 4]
                nn = 512 if j < NCV - 1 else 256
                nc.tensor.matmul(
                    cv[:, :nn],
                    wb_t[:, :],
                    p9[:, 512 * j : 512 * j + nn],
                    start=True,
                    stop=True,
                )
                ecopy(j, yt[:, 512 * j : 512 * j + nn], cv[:, :nn])

            # out pieces: 7x2080 + 2x1040; piece q ready after conv chunk
            OUT_PIECES = [(2080 * q, 2080) for q in range(7)] + [
                (14560, 1040),
                (15600, 1040),
            ]
            out_after = {4: 0, 8: 1, 12: 2, 16: 3, 20: 4, 24: 5, 28: 6, 30: 7, 32: 8}

            def emit_out(q):
                o, n = OUT_PIECES[q]
                nc.sync.dma_start(
                    out=y.ap()[:, o : o + n], in_=yt[:, o : o + n]
                )

            def emit_conv_full(j):
                emit_conv(j)
                if j in out_after:
                    emit_out(out_after[j])

            for k in range(32):
                emit_cs(k)
                if k in GCH:
                    emit_bounce(GCH.index(k))
            for j in range(NCV):
                emit_conv_full(j)
            if dump:
                nc.sync.dma_start(out=p9_d.ap()[:, :], in_=p9[:, 0:HHW])

    nc.compile()
    return nc


def _host_prep(x, weight):
    wsum = weight.sum(axis=1)  # [COUT, 3, 3]
    wb = np.zeros((NK, NOUT), np.float32)
    for b in range(BPC):
        for jj in range(3):
            for ip in range(3):
                wb[6 * jj + 2 * ip + b, b * COUT : (b + 1) * COUT] = wsum[
                    :, 2 - jj, 2 - ip
                ]
    wb = wb.astype(np.float16)

    in_maps = []
    for r in range(N_CORES):
        xhr = np.ascontiguousarray(
            x[r * BPC : (r + 1) * BPC].reshape(NPART, HW)
        ).astype(np.float16)
        in_maps.append({"xh": xhr, "wb": wb})
    return in_maps


def kernel(x, weight, bias):
    from concourse.bass_utils import run_bass_kernel_spmd

    x = np.asarray(x)
    weight = np.asarray(weight)
    bias = np.asarray(bias)
    nc = _build()
    in_maps = _host_prep(x, weight)
    res = run_bass_kernel_spmd(nc, in_maps, core_ids=list(range(N_CORES)))
    out = np.concatenate(
        [
            np.asarray(res.results[r]["y"])
            .astype(np.float32)
            .reshape(BPC, COUT, H, WROW)[:, :, :, :W]
            for r in range(N_CORES)
        ],
        axis=0,
    )
    return out + bias.astype(np.float32)[None, :, None, None]
